# revision 29
# baseline (speedup 1.0000x reference)
# DCNv2 (modulated deformable conv) Trainium2 Bass kernel.
#
# Sharding: pure data parallel over 8 cores; core = (batch, H-half), each
# core computes a (256, 32, 64) output slab from a zero-padded input slab.
#
# Per-core pipeline (preamble split in 2 halves interleaved with the
# tile loop):
#   1. offset/mask 3x3 conv on the PE (bf16 matmuls, fp32 PSUM, fused
#      bias (+tap/pad constants) and sigmoid on the ACT engine)
#   2. sampling fields (py/px/floor/frac/bilinear corner weights incl.
#      mask) on DVE in bf16; weights sigma-reordered
#   3. gather: SWDGE dma_gather from an HBM-resident 4-corner row table
#      (row l = [x[l], x[l+1], x[l+WP], x[l+WP+1]], 2KB) - ONE descriptor
#      per (tap, position)
#   4. 4-corner weighted combine split across engines: DVE multiplies
#      corners 0,1 (batched broadcast-AP op), ACT scales corners 2,3
#      in-place (per-partition scale), DVE does the 2-level add tree
#   5. colsT -> cols via a single XBAR DMA transpose per tile
#   6. main contraction out[o,hw] = sum_{c,p} w[o,c,p] cols[c,p,hw] as
#      bf16 matmuls accumulating in PSUM; bias on the ACT copy, un-sigma
#      in the output DMA APs.
import numpy as np
import ml_dtypes

import concourse.bass as bass
import concourse.mybir as mybir
from concourse import bacc
import concourse.tile as tile
from concourse import library_config
from concourse.masks import make_identity
from concourse import bass_utils

BF16 = ml_dtypes.bfloat16

B, C, H, W = 4, 256, 64, 64
O, K = 256, 3
KK = K * K
NCORES = 8
HH = H // 2            # 32 output rows per core
PAD = 5                # zero halo; requires |offset| < PAD - 1
HP, WP = 48, 80        # padded local input dims
L = HP * WP            # 3840 source locations
NHW = HH * W           # 2048 output positions per core
NT = NHW // 128        # 16 gather tiles
NHWH = NHW // 2        # positions per preamble half
CG = C // 128
OG = O // 128
A = mybir.AluOpType
ACTF = mybir.ActivationFunctionType
FP32 = mybir.dt.float32
BF = mybir.dt.bfloat16
I16 = mybir.dt.int16

MAGIC = float(np.float32(2 ** 23))


def build_nc():
    nc = bacc.Bacc("TRN2", target_bir_lowering=False, num_devices=NCORES,
                   num_swdge_queues=2)

    x_cm_d = nc.dram_tensor("x_cm", [CG, 128, HP, WP], BF, kind="ExternalInput").ap()
    xT4_d = nc.dram_tensor("xT4", [L, 4 * C], BF, kind="ExternalInput").ap()
    w_om_d = nc.dram_tensor("w_om", [128, KK, CG, 73], BF, kind="ExternalInput").ap()
    b_om_d = nc.dram_tensor("b_om", [73, 1], FP32, kind="ExternalInput").ap()
    w_mm_d = nc.dram_tensor("w_mm", [128, 4 * KK, OG, 128], BF,
                            kind="ExternalInput").ap()
    b_o_d = nc.dram_tensor("b_o", [128, OG, 1], FP32, kind="ExternalInput").ap()
    hio_d = nc.dram_tensor("hio", [KK, HH], BF, kind="ExternalInput").ap()
    wio_d = nc.dram_tensor("wio", [KK, W], BF, kind="ExternalInput").ap()
    y_d = nc.dram_tensor("y", [OG, 128, NHW], BF, kind="ExternalOutput").ap()

    with tile.TileContext(nc) as tc:
        with (
            tc.tile_pool(name="const", bufs=1) as const,
            tc.tile_pool(name="persist", bufs=1) as persist,
            tc.tile_pool(name="dramp", bufs=1, space="DRAM") as dramp,
            tc.tile_pool(name="fld", bufs=1) as fld,
            tc.tile_pool(name="cwp", bufs=16) as cwp,
            tc.tile_pool(name="qp", bufs=3) as qp,
            tc.tile_pool(name="hp", bufs=2) as hp,
            tc.tile_pool(name="colsp", bufs=2) as colsp,
            tc.tile_pool(name="outp", bufs=2) as outp,
            tc.tile_pool(name="ps_conv", bufs=1, space="PSUM") as ps_conv,
            tc.tile_pool(name="ps_cw", bufs=4, space="PSUM") as ps_cw,
            tc.tile_pool(name="ps_m", bufs=2, space="PSUM") as ps_m,
        ):
            # ---- constants into SBUF ----
            w_om = const.tile([128, KK, CG, 73], BF)
            nc.sync.dma_start(w_om[:], w_om_d)
            w_mm = const.tile([128, 4 * KK, OG, 128], BF)
            nc.scalar.dma_start(w_mm[:], w_mm_d)
            b_om = const.tile([73, 1], FP32)
            nc.sync.dma_start(b_om[:], b_om_d)
            b_o = const.tile([128, OG, 1], FP32)
            nc.sync.dma_start(b_o[:], b_o_d)
            hio2 = const.tile([KK, HH], BF)
            nc.sync.dma_start(hio2[:], hio_d)
            wio2 = const.tile([KK, W], BF)
            nc.sync.dma_start(wio2[:], wio_d)
            idbf = const.tile([128, 128], BF)
            make_identity(nc, idbf[:])
            nc.gpsimd.load_library(library_config.mlp)

            idx16 = persist.tile([KK, NHW], I16)
            # bilinear corner weights, sigma-reordered, bf16
            w00s = persist.tile([KK, NHW], BF)
            w01s = persist.tile([KK, NHW], BF)
            w10s = persist.tile([KK, NHW], BF)
            w11s = persist.tile([KK, NHW], BF)
            wrapped = persist.tile([128, NT, 72], I16)
            db = dramp.tile([16, NT, 72], I16, name="db")

            # x slab rows 4..34 are all the conv reads (out rows 0..31,
            # taps 0..2, offset +4-1); two half tiles of 15+16 rows
            x_sb = []
            for cg in range(CG):
                xa = fld.tile([128, 19, WP], BF, name=f"xsbA{cg}")
                nc.sync.dma_start(xa[:], x_cm_d[cg][:, 4:23])
                xb = fld.tile([128, 19, WP], BF, name=f"xsbB{cg}")
                nc.scalar.dma_start(xb[:], x_cm_d[cg][:, 20:39])
                x_sb.append((xa, xb))

            corners = [w00s, w01s, w10s, w11s]

            def emit_half(hf):
                """offset/mask conv + sampling fields + idx bounce for
                output rows [hf*16, hf*16+16) = tiles [8hf, 8hf+8).
                Returns a list of emission chunks to interleave."""
                cols = slice(hf * NHWH, (hf + 1) * NHWH)
                st = {}

                def c_alloc():
                    st["offy"] = fld.tile([KK, NHWH], FP32, tag="t_f32",
                                          bufs=3, name=f"offy{hf}")
                    st["offx"] = fld.tile([KK, NHWH], FP32, tag="t_f32",
                                          bufs=3, name=f"offx{hf}")
                    st["msk"] = fld.tile([KK, NHWH], BF, tag="t_msk", bufs=2,
                                         name=f"msk{hf}")

                def c_conv(bl):
                    offy_h, offx_h, msk = st["offy"], st["offx"], st["msk"]
                    bk = hf * 4 + bl
                    ps = ps_conv.tile([73, 4 * WP], FP32, tag="psc")
                    n = 0
                    for cg in range(CG):
                        for tap in range(KK):
                            ky, kx = tap // K, tap % K
                            r = bk * 4 + ky - 16 * hf
                            xt = x_sb[cg][hf]
                            rhs = xt[:, r, 4 + kx:]
                            rhs = bass.AP(tensor=rhs.tensor, offset=rhs.offset,
                                          ap=[rhs.ap[0], [1, 4 * WP]])
                            nc.tensor.matmul(ps[:], w_om[:, tap, cg], rhs,
                                             start=(n == 0),
                                             stop=(n == 2 * KK - 1))
                            n += 1
                    sl = slice(bl * 4 * W, (bl + 1) * 4 * W)
                    psv = [None, None, None]
                    for i, base in enumerate((0, 32, 64)):
                        p4 = ps[base:base + 9].rearrange("c (r x) -> c r x",
                                                         r=4)
                        psv[i] = p4[:, :, 0:W]
                    nc.scalar.activation(offy_h[:, sl], psv[0], ACTF.Identity,
                                         bias=b_om[0:9])
                    nc.scalar.activation(offx_h[:, sl], psv[1], ACTF.Identity,
                                         bias=b_om[32:41])
                    nc.scalar.activation(msk[:, sl], psv[2], ACTF.Sigmoid,
                                         bias=b_om[64:73])

                def c_fields_a():
                    offy_h, offx_h = st["offy"], st["offx"]
                    # sampling positions: py = offy + h, px = offx + w
                    # (broadcast row/col index via stride-0 APs)
                    py = fld.tile([KK, NHWH], FP32, tag="t_f32", bufs=3,
                                  name="py")
                    px = fld.tile([KK, NHWH], FP32, tag="t_f32", bufs=3,
                                  name="px")
                    st["py"], st["px"] = py, px
                    hbc = bass.AP(tensor=hio2.tensor,
                                  offset=hio2.offset + hf * 16,
                                  ap=[hio2[:].ap[0], [1, 16], [0, W]])
                    wbc2 = bass.AP(tensor=wio2.tensor, offset=wio2.offset,
                                   ap=[wio2[:].ap[0], [0, 16], [1, W]])
                    ov = offy_h[:].rearrange("c (h w) -> c h w", h=16)
                    xv = offx_h[:].rearrange("c (h w) -> c h w", h=16)
                    nc.vector.tensor_tensor(
                        py[:].rearrange("c (h w) -> c h w", h=16), ov, hbc,
                        A.add)
                    nc.vector.tensor_tensor(
                        px[:].rearrange("c (h w) -> c h w", h=16), xv, wbc2,
                        A.add)

                    def floor_clamp(dst, src, hi):
                        # dst = floor(src) via round(src-0.5); offsets are
                        # bounded (|off|<4 asserted) so no clamp is needed
                        t1 = fld.tile([KK, NHWH], FP32, tag="t_fc", bufs=1,
                                      name="fc1")
                        nc.vector.tensor_scalar(t1[:], src[:], MAGIC - 0.5,
                                                None, A.add)
                        nc.vector.tensor_scalar(dst[:], t1[:], MAGIC, None,
                                                A.subtract)

                    y0 = fld.tile([KK, NHWH], BF, tag="t_bf", bufs=3,
                                  name="y0")
                    x0 = fld.tile([KK, NHWH], BF, tag="t_bf", bufs=3,
                                  name="x0")
                    st["y0"], st["x0"] = y0, x0
                    floor_clamp(y0, py, HP - 2)
                    floor_clamp(x0, px, WP - 2)
                    nc.vector.scalar_tensor_tensor(idx16[:, cols], y0[:],
                                                   float(WP), x0[:],
                                                   A.mult, A.add)

                def c_fields_b():
                    py, px = st["py"], st["px"]
                    y0, x0, msk = st["y0"], st["x0"], st["msk"]
                    fy = fld.tile([KK, NHWH], BF, tag="t_bf", bufs=3,
                                  name="fy")
                    nc.vector.tensor_tensor(fy[:], py[:], y0[:], A.subtract)
                    fx = fld.tile([KK, NHWH], BF, tag="t_bf", bufs=3,
                                  name="fx")
                    nc.vector.tensor_tensor(fx[:], px[:], x0[:], A.subtract)

                    u = fld.tile([KK, NHWH], BF, tag="t_ug", bufs=2, name="u")
                    gy = fld.tile([KK, NHWH], BF, tag="t_ug", bufs=2,
                                  name="gy")
                    nc.vector.tensor_tensor(u[:], fy[:], msk[:], A.mult)
                    nc.vector.tensor_tensor(gy[:], msk[:], u[:], A.subtract)

                    # w01 = gy*fx; w00 = gy-w01; w11 = u*fx; w10 = u-w11
                    wna = fld.tile([KK, NHWH], BF, tag="t_wn", bufs=2,
                                   name="wna")
                    wnb = fld.tile([KK, NHWH], BF, tag="t_wn", bufs=2,
                                   name="wnb")

                    def sigma_store(wdst, wsrc):
                        src = wsrc[:].rearrange("c (t p a) -> c t p a", t=8,
                                                p=16).transpose([0, 1, 3, 2])
                        nc.vector.tensor_copy(wdst[:, cols], src)

                    nc.vector.tensor_tensor(wna[:], gy[:], fx[:], A.mult)
                    nc.vector.tensor_tensor(wnb[:], gy[:], wna[:], A.subtract)
                    sigma_store(w01s, wna)
                    sigma_store(w00s, wnb)
                    wnc = fld.tile([KK, NHWH], BF, tag="t_wn", bufs=2,
                                   name="wnc")
                    wnd = fld.tile([KK, NHWH], BF, tag="t_wn", bufs=2,
                                   name="wnd")
                    nc.vector.tensor_tensor(wnc[:], u[:], fx[:], A.mult)
                    nc.vector.tensor_tensor(wnd[:], u[:], wnc[:], A.subtract)
                    sigma_store(w11s, wnc)
                    sigma_store(w10s, wnd)

                def c_cw():
                    # per-position corner weights for this half's 8 tiles
                    # (hoisted off the per-tile path so the ACT queue never
                    # blocks DVE's combine on a stalled out-copy)
                    for tt in range(8):
                        t = hf * 8 + tt
                        cw4T = cwp.tile([128, KK, 4], BF, tag="cw4T",
                                        name="cw4T")
                        cw_all[t] = cw4T
                        for j, wf in enumerate(corners):
                            psf = ps_cw.tile([128, KK], BF, tag="psf",
                                             name="psf")
                            nc.tensor.transpose(
                                psf[:], wf[0:9, t * 128:(t + 1) * 128],
                                idbf[0:9, 0:9])
                            dst = bass.AP(tensor=cw4T.tensor,
                                          offset=cw4T.offset + j,
                                          ap=[cw4T[:].ap[0], [4, KK]])
                            nc.scalar.activation(dst, psf[:], ACTF.Identity)

                def c_bounce():
                    # idx -> DRAM bounce (partition-major) -> replicated
                    for tt in range(8):
                        t = hf * 8 + tt
                        dst_ap = bass.AP(tensor=db.tensor,
                                         offset=db.offset + t * 72,
                                         ap=[[8, KK], [NT * 72, 16], [1, 8]])
                        src_w = idx16[:, t * 128:(t + 1) * 128].rearrange(
                            "b (p q) -> b p q", p=16)
                        nc.sync.dma_start(dst_ap, src_w)
                    rep_ap = bass.AP(tensor=db.tensor,
                                     offset=db.offset + hf * 8 * 72,
                                     ap=[[0, 8], [NT * 72, 16], [1, 8 * 72]])
                    wdst = wrapped[:].rearrange("p t q -> p (t q)")
                    nc.sync.dma_start(wdst[:, hf * 576:(hf + 1) * 576],
                                      rep_ap)

                def c_cw_bounce():
                    c_cw()
                    c_bounce()

                return [c_alloc, lambda: c_conv(0), lambda: c_conv(1),
                        lambda: c_conv(2), lambda: c_conv(3),
                        c_fields_a, c_fields_b, c_cw_bounce]

            def emit_tile(t, cols_sb):
                cw4T = cw_all[t]
                # gather Q4[128(sig hw), 9(tap), 1024(4 corners x 256c)]
                Q4 = qp.tile([128, KK, C, 4], BF, tag="Q4", name="Q4")
                nc.gpsimd.dma_gather(
                    out_ap=Q4[:].rearrange("p k c j -> p k (c j)"),
                    in_ap=xT4_d, idxs_ap=wrapped[:, t],
                    num_idxs=KK * 128, num_idxs_reg=KK * 128,
                    elem_size=4 * C, single_packet=False, queue_num=t % 2,
                )

                # weighted 4-corner combine, all on DVE with stride-1 last
                # dims (keeps the 16-bit 2x fast path): corners innermost
                p0 = Q4[:].ap[0]
                wbc = bass.AP(tensor=cw4T.tensor, offset=cw4T.offset,
                              ap=[cw4T[:].ap[0], [4, KK], [0, C], [1, 4]])
                nc.vector.tensor_tensor(Q4[:], Q4[:], wbc, A.mult)

                # add tree: h2[.,c,y'] = y-pairs; colsT = x-pairs
                h2 = hp.tile([128, KK, C, 2], BF, tag="h", name="h")
                qe = bass.AP(tensor=Q4.tensor, offset=Q4.offset,
                             ap=[p0, [4 * C, KK], [4, C], [1, 2]])
                qo = bass.AP(tensor=Q4.tensor, offset=Q4.offset + 2,
                             ap=[p0, [4 * C, KK], [4, C], [1, 2]])
                nc.vector.tensor_tensor(h2[:], qe, qo, A.add)

                # transpose h2 to cols2 [128(2c+j), 36 blk, 256(sig)]
                # via XBAR; the x-corner sum rides the main contraction
                csrc = bass.AP(tensor=h2.tensor, offset=h2.offset,
                               ap=[h2[:].ap[0], [1, KK * C * 2]])
                cdst = bass.AP(tensor=cols_sb.tensor,
                               offset=cols_sb.offset + (t % 2) * 128,
                               ap=[cols_sb[:].ap[0], [256, 4 * KK], [1, 128]])
                nc.sync.dma_start(cdst, csrc, transpose=True)

            def emit_group(g, cols_sb):
                # main contraction + bias + store for tiles 2g..2g+1
                for og in range(OG):
                    psO = ps_m.tile([128, 256], FP32, tag="psO", name="psO")
                    for b in range(4 * KK):
                        nc.tensor.matmul(
                            psO[:], w_mm[:, b, og], cols_sb[:, b],
                            start=(b == 0), stop=(b == 4 * KK - 1))
                    out_t = outp.tile([128, 256], BF, tag="out", name="out_t")
                    nc.scalar.activation(out_t[:], psO[:], ACTF.Identity,
                                         bias=b_o[:, og])
                    # stored in sigma order; host un-permutes
                    nc.scalar.dma_start(y_d[og, :, g * 256:(g + 1) * 256],
                                        out_t[:])

            cw_all = [None] * NT
            # ---- emission: half 0 upfront; half 1 spread over tiles ----
            for c in emit_half(0):
                c()
            h1 = emit_half(1)
            h1[0]()
            cols_sb = None
            for t in range(NT):
                if 0 <= t < 7:
                    h1[t + 1]()
                if t % 2 == 0:
                    cols_sb = colsp.tile([128, 4 * KK, 256], BF, tag="cols",
                                         name="cols")
                emit_tile(t, cols_sb)
                if t % 2 == 1:
                    emit_group(t // 2, cols_sb)
    nc.compile()
    return nc


# ---------------- host side ----------------

def host_prep(x, w_off, b_off, w_mask, b_mask, w_dcn, b_dcn):
    """Build the 8 per-core input maps (pure layout prep / sharding)."""
    x = np.asarray(x, np.float32)
    w_off = np.asarray(w_off, np.float32)
    w_mask = np.asarray(w_mask, np.float32)
    b_off = np.asarray(b_off, np.float32)
    b_mask = np.asarray(b_mask, np.float32)
    w_dcn = np.asarray(w_dcn, np.float32)
    b_dcn = np.asarray(b_dcn, np.float32)

    wcat = np.zeros((73, C, K, K), np.float32)
    wcat[0:9] = w_off[0::2]
    wcat[32:41] = w_off[1::2]
    wcat[64:73] = w_mask
    w_om = np.zeros((128, KK, CG, 73), BF16)
    for tap in range(KK):
        ky, kx = tap // K, tap % K
        for cg in range(CG):
            w_om[:, tap, cg] = (
                wcat[:, cg * 128:(cg + 1) * 128, ky, kx].T.astype(BF16))

    ky_t = np.repeat(np.arange(K), K).astype(np.float32)
    kx_t = np.tile(np.arange(K), K).astype(np.float32)
    b_om = np.zeros((73, 1), np.float32)
    b_om[0:9, 0] = b_off[0::2] + ky_t - 1 + PAD
    b_om[32:41, 0] = b_off[1::2] + kx_t - 1 + PAD
    b_om[64:73, 0] = b_mask

    # doubled-contraction layout: XBAR'd h2 block b = (tap=b//4,
    # 64-chan segment b%4), partition q = 2*(c - 64*(b%4)) + x-corner;
    # both x-corners multiply the same weight (their sum = the sample)
    w_mm = np.zeros((128, 4 * KK, OG, 128), BF16)
    for tap in range(KK):
        ky, kx = tap // K, tap % K
        for seg in range(4):
            b = tap * 4 + seg
            cs = seg * 64
            wblk = w_dcn[:, cs:cs + 64, ky, kx].astype(BF16)  # [O, 64]
            for og in range(OG):
                wo = wblk[og * 128:(og + 1) * 128].T  # [64, 128]
                w_mm[0::2, b, og] = wo
                w_mm[1::2, b, og] = wo
    b_o = b_dcn.reshape(OG, 128, 1).transpose(1, 0, 2).copy()

    hio = np.broadcast_to(np.arange(HH, dtype=np.float32)[None, :],
                          (KK, HH)).astype(BF16)
    wio = np.broadcast_to(np.arange(W, dtype=np.float32)[None, :],
                          (KK, W)).astype(BF16)

    shared = dict(w_om=w_om, b_om=b_om, w_mm=w_mm, b_o=b_o, hio=hio, wio=wio)

    in_maps = []
    for core in range(NCORES):
        b, half = core // 2, core % 2
        h0 = half * HH
        xp = np.zeros((C, HP, WP), np.float32)
        glo, ghi = h0 - PAD, h0 + HH + PAD
        slo, shi = max(glo, 0), min(ghi, H)
        xp[:, slo - glo: slo - glo + (shi - slo), PAD:PAD + W] = x[b, :, slo:shi, :]
        xbf = xp.astype(BF16)
        x_cm = np.ascontiguousarray(xbf.reshape(CG, 128, HP, WP))
        xT = np.ascontiguousarray(xbf.reshape(C, L).T)  # [L, C]
        xT4c = np.zeros((4, L, C), BF16)
        xT4c[0] = xT
        xT4c[1, :L - 1] = xT[1:]
        xT4c[2, :L - WP] = xT[WP:]
        xT4c[3, :L - WP - 1] = xT[WP + 1:]
        # interleave corners per channel: row l = [c0 x 4 corners, c1 x ...]
        xT4 = np.ascontiguousarray(
            xT4c.transpose(1, 2, 0).reshape(L, 4 * C))
        im = dict(shared)
        im["x_cm"] = x_cm
        im["xT4"] = xT4
        in_maps.append(im)
    return in_maps


_NC_CACHE = {}


def kernel(**inputs):
    if "nc" not in _NC_CACHE:
        _NC_CACHE["nc"] = build_nc()
    nc = _NC_CACHE["nc"]
    in_maps = host_prep(**inputs)
    res = bass_utils.run_bass_kernel_spmd(nc, in_maps,
                                          core_ids=list(range(NCORES)))
    out = np.zeros((B, O, H, W), np.float32)
    for core in range(NCORES):
        b, half = core // 2, core % 2
        yv = np.asarray(res.results[core]["y"], np.float32)
        # un-sigma: column t*128 + a*16 + p holds position t*128 + p*8 + a
        yv = yv.reshape(O, NT, 8, 16).transpose(0, 1, 3, 2).reshape(O, HH, W)
        out[b, :, half * HH:(half + 1) * HH, :] = yv
    return out


# revision 31
# speedup vs baseline: 1.0337x; 1.0337x over previous
# DCNv2 (modulated deformable conv) Trainium2 Bass kernel.
#
# Sharding: pure data parallel over 8 cores; core = (batch, H-half), each
# core computes a (256, 32, 64) output slab from a zero-padded input slab.
#
# Per-core pipeline (preamble split in 2 halves interleaved with the
# tile loop):
#   1. offset/mask 3x3 conv on the PE (bf16 matmuls, fp32 PSUM, fused
#      bias (+tap/pad constants) and sigmoid on the ACT engine)
#   2. sampling fields (py/px/floor/frac/bilinear corner weights incl.
#      mask) on DVE in bf16; weights sigma-reordered
#   3. gather: SWDGE dma_gather from an HBM-resident 4-corner row table
#      (row l = [x[l], x[l+1], x[l+WP], x[l+WP+1]], 2KB) - ONE descriptor
#      per (tap, position)
#   4. 4-corner weighted combine split across engines: DVE multiplies
#      corners 0,1 (batched broadcast-AP op), ACT scales corners 2,3
#      in-place (per-partition scale), DVE does the 2-level add tree
#   5. colsT -> cols via a single XBAR DMA transpose per tile
#   6. main contraction out[o,hw] = sum_{c,p} w[o,c,p] cols[c,p,hw] as
#      bf16 matmuls accumulating in PSUM; bias on the ACT copy, un-sigma
#      in the output DMA APs.
import numpy as np
import ml_dtypes

import concourse.bass as bass
import concourse.mybir as mybir
from concourse import bacc
import concourse.tile as tile
from concourse import library_config
from concourse.masks import make_identity
from concourse import bass_utils

BF16 = ml_dtypes.bfloat16

B, C, H, W = 4, 256, 64, 64
O, K = 256, 3
KK = K * K
NCORES = 8
HH = H // 2            # 32 output rows per core
PAD = 5                # zero halo; requires |offset| < PAD - 1
HP, WP = 48, 80        # padded local input dims
L = HP * WP            # 3840 source locations
NHW = HH * W           # 2048 output positions per core
NT = NHW // 128        # 16 gather tiles
NHWH = NHW // 2        # positions per preamble half
CG = C // 128
OG = O // 128
A = mybir.AluOpType
ACTF = mybir.ActivationFunctionType
FP32 = mybir.dt.float32
BF = mybir.dt.bfloat16
I16 = mybir.dt.int16

MAGIC = float(np.float32(2 ** 23))


def build_nc():
    nc = bacc.Bacc("TRN2", target_bir_lowering=False, num_devices=NCORES,
                   num_swdge_queues=2)

    x_cm_d = nc.dram_tensor("x_cm", [CG, 128, HP, WP], BF, kind="ExternalInput").ap()
    xT4_d = nc.dram_tensor("xT4", [L, 4 * C], BF, kind="ExternalInput").ap()
    w_om_d = nc.dram_tensor("w_om", [128, KK, CG, 73], BF, kind="ExternalInput").ap()
    b_om_d = nc.dram_tensor("b_om", [73, 1], FP32, kind="ExternalInput").ap()
    w_mm_d = nc.dram_tensor("w_mm", [128, 4 * KK, OG, 128], BF,
                            kind="ExternalInput").ap()
    b_o_d = nc.dram_tensor("b_o", [128, OG, 1], FP32, kind="ExternalInput").ap()
    hio_d = nc.dram_tensor("hio", [KK, HH], BF, kind="ExternalInput").ap()
    wio_d = nc.dram_tensor("wio", [KK, W], BF, kind="ExternalInput").ap()
    y_d = nc.dram_tensor("y", [OG, 128, NHW], BF, kind="ExternalOutput").ap()

    with tile.TileContext(nc) as tc:
        with (
            tc.tile_pool(name="const", bufs=1) as const,
            tc.tile_pool(name="persist", bufs=1) as persist,
            tc.tile_pool(name="dramp", bufs=1, space="DRAM") as dramp,
            tc.tile_pool(name="fld", bufs=1) as fld,
            tc.tile_pool(name="cwp", bufs=16) as cwp,
            tc.tile_pool(name="qp", bufs=3) as qp,
            tc.tile_pool(name="hp", bufs=2) as hp,
            tc.tile_pool(name="colsp", bufs=2) as colsp,
            tc.tile_pool(name="outp", bufs=2) as outp,
            tc.tile_pool(name="ps_conv", bufs=1, space="PSUM") as ps_conv,
            tc.tile_pool(name="ps_cw", bufs=2, space="PSUM") as ps_cw,
            tc.tile_pool(name="ps_m", bufs=4, space="PSUM") as ps_m,
        ):
            # ---- constants into SBUF ----
            w_om = const.tile([128, KK, CG, 73], BF)
            nc.sync.dma_start(w_om[:], w_om_d)
            w_mm = const.tile([128, 4 * KK, OG, 128], BF)
            nc.scalar.dma_start(w_mm[:], w_mm_d)
            b_om = const.tile([73, 1], FP32)
            nc.sync.dma_start(b_om[:], b_om_d)
            b_o = const.tile([128, OG, 1], FP32)
            nc.sync.dma_start(b_o[:], b_o_d)
            hio2 = const.tile([KK, HH], BF)
            nc.sync.dma_start(hio2[:], hio_d)
            wio2 = const.tile([KK, W], BF)
            nc.sync.dma_start(wio2[:], wio_d)
            idbf = const.tile([128, 128], BF)
            make_identity(nc, idbf[:])
            nc.gpsimd.load_library(library_config.mlp)

            idx16 = persist.tile([KK, NHW], I16)
            # bilinear corner weights, sigma-reordered, bf16
            w00s = persist.tile([KK, NHW], BF)
            w01s = persist.tile([KK, NHW], BF)
            w10s = persist.tile([KK, NHW], BF)
            w11s = persist.tile([KK, NHW], BF)
            wrapped = persist.tile([128, NT, 72], I16)
            db = dramp.tile([16, NT, 72], I16, name="db")

            # x slab rows 4..34 are all the conv reads (out rows 0..31,
            # taps 0..2, offset +4-1); two half tiles of 15+16 rows
            x_sb = []
            for cg in range(CG):
                xa = fld.tile([128, 19, WP], BF, name=f"xsbA{cg}")
                nc.sync.dma_start(xa[:], x_cm_d[cg][:, 4:23])
                xb = fld.tile([128, 19, WP], BF, name=f"xsbB{cg}")
                nc.scalar.dma_start(xb[:], x_cm_d[cg][:, 20:39])
                x_sb.append((xa, xb))

            corners = [w00s, w01s, w10s, w11s]

            def emit_half(hf):
                """offset/mask conv + sampling fields + idx bounce for
                output rows [hf*16, hf*16+16) = tiles [8hf, 8hf+8).
                Returns a list of emission chunks to interleave."""
                cols = slice(hf * NHWH, (hf + 1) * NHWH)
                st = {}

                def c_alloc():
                    st["offy"] = fld.tile([KK, NHWH], FP32, tag="t_f32",
                                          bufs=3, name=f"offy{hf}")
                    st["offx"] = fld.tile([KK, NHWH], FP32, tag="t_f32",
                                          bufs=3, name=f"offx{hf}")
                    st["msk"] = fld.tile([KK, NHWH], BF, tag="t_msk", bufs=2,
                                         name=f"msk{hf}")

                def c_conv(bl):
                    offy_h, offx_h, msk = st["offy"], st["offx"], st["msk"]
                    bk = hf * 4 + bl
                    ps = ps_conv.tile([73, 4 * WP], FP32, tag="psc")
                    n = 0
                    for cg in range(CG):
                        for tap in range(KK):
                            ky, kx = tap // K, tap % K
                            r = bk * 4 + ky - 16 * hf
                            xt = x_sb[cg][hf]
                            rhs = xt[:, r, 4 + kx:]
                            rhs = bass.AP(tensor=rhs.tensor, offset=rhs.offset,
                                          ap=[rhs.ap[0], [1, 4 * WP]])
                            nc.tensor.matmul(ps[:], w_om[:, tap, cg], rhs,
                                             start=(n == 0),
                                             stop=(n == 2 * KK - 1))
                            n += 1
                    sl = slice(bl * 4 * W, (bl + 1) * 4 * W)
                    psv = [None, None, None]
                    for i, base in enumerate((0, 32, 64)):
                        p4 = ps[base:base + 9].rearrange("c (r x) -> c r x",
                                                         r=4)
                        psv[i] = p4[:, :, 0:W]
                    nc.scalar.activation(offy_h[:, sl], psv[0], ACTF.Identity,
                                         bias=b_om[0:9])
                    nc.scalar.activation(offx_h[:, sl], psv[1], ACTF.Identity,
                                         bias=b_om[32:41])
                    nc.scalar.activation(msk[:, sl], psv[2], ACTF.Sigmoid,
                                         bias=b_om[64:73])

                def c_fields_a():
                    offy_h, offx_h = st["offy"], st["offx"]
                    # sampling positions: py = offy + h, px = offx + w
                    # (broadcast row/col index via stride-0 APs)
                    py = fld.tile([KK, NHWH], FP32, tag="t_f32", bufs=3,
                                  name="py")
                    px = fld.tile([KK, NHWH], FP32, tag="t_f32", bufs=3,
                                  name="px")
                    st["py"], st["px"] = py, px
                    hbc = bass.AP(tensor=hio2.tensor,
                                  offset=hio2.offset + hf * 16,
                                  ap=[hio2[:].ap[0], [1, 16], [0, W]])
                    wbc2 = bass.AP(tensor=wio2.tensor, offset=wio2.offset,
                                   ap=[wio2[:].ap[0], [0, 16], [1, W]])
                    ov = offy_h[:].rearrange("c (h w) -> c h w", h=16)
                    xv = offx_h[:].rearrange("c (h w) -> c h w", h=16)
                    nc.vector.tensor_tensor(
                        py[:].rearrange("c (h w) -> c h w", h=16), ov, hbc,
                        A.add)
                    nc.vector.tensor_tensor(
                        px[:].rearrange("c (h w) -> c h w", h=16), xv, wbc2,
                        A.add)

                    def floor_clamp(dst, src, hi):
                        # dst = floor(src) via round(src-0.5); offsets are
                        # bounded (|off|<4 asserted) so no clamp is needed
                        t1 = fld.tile([KK, NHWH], FP32, tag="t_fc", bufs=1,
                                      name="fc1")
                        nc.vector.tensor_scalar(t1[:], src[:], MAGIC - 0.5,
                                                None, A.add)
                        nc.vector.tensor_scalar(dst[:], t1[:], MAGIC, None,
                                                A.subtract)

                    y0 = fld.tile([KK, NHWH], BF, tag="t_bf", bufs=3,
                                  name="y0")
                    x0 = fld.tile([KK, NHWH], BF, tag="t_bf", bufs=3,
                                  name="x0")
                    st["y0"], st["x0"] = y0, x0
                    floor_clamp(y0, py, HP - 2)
                    floor_clamp(x0, px, WP - 2)
                    nc.vector.scalar_tensor_tensor(idx16[:, cols], y0[:],
                                                   float(WP), x0[:],
                                                   A.mult, A.add)

                def c_fields_b():
                    py, px = st["py"], st["px"]
                    y0, x0, msk = st["y0"], st["x0"], st["msk"]
                    fy = fld.tile([KK, NHWH], BF, tag="t_bf", bufs=3,
                                  name="fy")
                    nc.vector.tensor_tensor(fy[:], py[:], y0[:], A.subtract)
                    fx = fld.tile([KK, NHWH], BF, tag="t_bf", bufs=3,
                                  name="fx")
                    nc.vector.tensor_tensor(fx[:], px[:], x0[:], A.subtract)

                    u = fld.tile([KK, NHWH], BF, tag="t_ug", bufs=2, name="u")
                    gy = fld.tile([KK, NHWH], BF, tag="t_ug", bufs=2,
                                  name="gy")
                    nc.vector.tensor_tensor(u[:], fy[:], msk[:], A.mult)
                    nc.vector.tensor_tensor(gy[:], msk[:], u[:], A.subtract)

                    # w01 = gy*fx; w00 = gy-w01; w11 = u*fx; w10 = u-w11
                    wna = fld.tile([KK, NHWH], BF, tag="t_wn", bufs=2,
                                   name="wna")
                    wnb = fld.tile([KK, NHWH], BF, tag="t_wn", bufs=2,
                                   name="wnb")

                    def sigma_store(wdst, wsrc):
                        src = wsrc[:].rearrange("c (t p a) -> c t p a", t=8,
                                                p=16).transpose([0, 1, 3, 2])
                        nc.vector.tensor_copy(wdst[:, cols], src)

                    nc.vector.tensor_tensor(wna[:], gy[:], fx[:], A.mult)
                    nc.vector.tensor_tensor(wnb[:], gy[:], wna[:], A.subtract)
                    sigma_store(w01s, wna)
                    sigma_store(w00s, wnb)
                    wnc = fld.tile([KK, NHWH], BF, tag="t_wn", bufs=2,
                                   name="wnc")
                    wnd = fld.tile([KK, NHWH], BF, tag="t_wn", bufs=2,
                                   name="wnd")
                    nc.vector.tensor_tensor(wnc[:], u[:], fx[:], A.mult)
                    nc.vector.tensor_tensor(wnd[:], u[:], wnc[:], A.subtract)
                    sigma_store(w11s, wnc)
                    sigma_store(w10s, wnd)

                def c_cw():
                    # per-position corner weights for this half's 8 tiles
                    # (hoisted off the per-tile path so the ACT queue never
                    # blocks DVE's combine on a stalled out-copy)
                    for tt in range(8):
                        t = hf * 8 + tt
                        cw4T = cwp.tile([128, KK, 4], BF, tag="cw4T",
                                        name="cw4T")
                        cw_all[t] = cw4T
                        for j, wf in enumerate(corners):
                            psf = ps_cw.tile([128, KK], BF, tag="psf",
                                             name="psf")
                            nc.tensor.transpose(
                                psf[:], wf[0:9, t * 128:(t + 1) * 128],
                                idbf[0:9, 0:9])
                            dst = bass.AP(tensor=cw4T.tensor,
                                          offset=cw4T.offset + j,
                                          ap=[cw4T[:].ap[0], [4, KK]])
                            nc.scalar.activation(dst, psf[:], ACTF.Identity)

                def c_bounce():
                    # idx -> DRAM bounce (partition-major) -> replicated
                    for tt in range(8):
                        t = hf * 8 + tt
                        dst_ap = bass.AP(tensor=db.tensor,
                                         offset=db.offset + t * 72,
                                         ap=[[8, KK], [NT * 72, 16], [1, 8]])
                        src_w = idx16[:, t * 128:(t + 1) * 128].rearrange(
                            "b (p q) -> b p q", p=16)
                        nc.sync.dma_start(dst_ap, src_w)
                    rep_ap = bass.AP(tensor=db.tensor,
                                     offset=db.offset + hf * 8 * 72,
                                     ap=[[0, 8], [NT * 72, 16], [1, 8 * 72]])
                    wdst = wrapped[:].rearrange("p t q -> p (t q)")
                    nc.sync.dma_start(wdst[:, hf * 576:(hf + 1) * 576],
                                      rep_ap)

                def c_cw_bounce():
                    c_cw()
                    c_bounce()

                return [c_alloc, lambda: c_conv(0), lambda: c_conv(1),
                        lambda: c_conv(2), lambda: c_conv(3),
                        c_fields_a, c_fields_b, c_cw_bounce]

            def emit_tile(t, cols_sb):
                cw4T = cw_all[t]
                # gather Q4[128(sig hw), 9(tap), 1024(4 corners x 256c)]
                Q4 = qp.tile([128, KK, C, 4], BF, tag="Q4", name="Q4")
                nc.gpsimd.dma_gather(
                    out_ap=Q4[:].rearrange("p k c j -> p k (c j)"),
                    in_ap=xT4_d, idxs_ap=wrapped[:, t],
                    num_idxs=KK * 128, num_idxs_reg=KK * 128,
                    elem_size=4 * C, single_packet=False, queue_num=t % 2,
                )

                # weighted 4-corner combine, all on DVE with stride-1 last
                # dims (keeps the 16-bit 2x fast path): corners innermost
                p0 = Q4[:].ap[0]
                wbc = bass.AP(tensor=cw4T.tensor, offset=cw4T.offset,
                              ap=[cw4T[:].ap[0], [4, KK], [0, C], [1, 4]])
                nc.vector.tensor_tensor(Q4[:], Q4[:], wbc, A.mult)

                # add tree: h2[.,c,y'] = y-pairs; colsT = x-pairs
                h2 = hp.tile([128, KK, C, 2], BF, tag="h", name="h")
                qe = bass.AP(tensor=Q4.tensor, offset=Q4.offset,
                             ap=[p0, [4 * C, KK], [4, C], [1, 2]])
                qo = bass.AP(tensor=Q4.tensor, offset=Q4.offset + 2,
                             ap=[p0, [4 * C, KK], [4, C], [1, 2]])
                nc.vector.tensor_tensor(h2[:], qe, qo, A.add)

                # transpose h2 to cols2 [128(2c+j), 36 blk, 256(sig)]
                # via XBAR; the x-corner sum rides the main contraction
                csrc = bass.AP(tensor=h2.tensor, offset=h2.offset,
                               ap=[h2[:].ap[0], [1, KK * C * 2]])
                cdst = bass.AP(tensor=cols_sb.tensor,
                               offset=cols_sb.offset + (t % 2) * 128,
                               ap=[cols_sb[:].ap[0], [256, 4 * KK], [1, 128]])
                nc.sync.dma_start(cdst, csrc, transpose=True)

            def emit_group(g, cols_sb, ps_out):
                # main contraction for tiles 2g..2g+1; result parked in
                # PSUM -- the bias+copy+store runs after the tile loop so
                # the ACT queue never head-of-line-blocks the combine
                for og in range(OG):
                    psO = ps_m.tile([128, 256], FP32, tag=f"psO{g % 2}",
                                    bufs=2, name="psO")
                    ps_out[g * OG + og] = psO
                    for b in range(4 * KK):
                        nc.tensor.matmul(
                            psO[:], w_mm[:, b, og], cols_sb[:, b],
                            start=(b == 0), stop=(b == 4 * KK - 1))

            def emit_store(g, ps_out):
                for og in range(OG):
                    psO = ps_out[g * OG + og]
                    out_t = outp.tile([128, 256], BF, tag="out", name="out_t")
                    nc.scalar.activation(out_t[:], psO[:], ACTF.Identity,
                                         bias=b_o[:, og])
                    # stored in sigma order; host un-permutes
                    nc.scalar.dma_start(y_d[og, :, g * 256:(g + 1) * 256],
                                        out_t[:])

            cw_all = [None] * NT
            ps_out = [None] * (8 * OG)
            # ---- emission: half 0 upfront; half 1 spread over tiles ----
            for c in emit_half(0):
                c()
            h1 = emit_half(1)
            h1[0]()
            cols_sb = None
            for t in range(NT):
                if 0 <= t < 7:
                    h1[t + 1]()
                if t % 2 == 0:
                    cols_sb = colsp.tile([128, 4 * KK, 256], BF, tag="cols",
                                         name="cols")
                emit_tile(t, cols_sb)
                if t % 2 == 1:
                    g = t // 2
                    emit_group(g, cols_sb, ps_out)
                    if g > 0:
                        emit_store(g - 1, ps_out)
                    if g == 7:
                        emit_store(7, ps_out)
    nc.compile()
    return nc


# ---------------- host side ----------------

def host_prep(x, w_off, b_off, w_mask, b_mask, w_dcn, b_dcn):
    """Build the 8 per-core input maps (pure layout prep / sharding)."""
    x = np.asarray(x, np.float32)
    w_off = np.asarray(w_off, np.float32)
    w_mask = np.asarray(w_mask, np.float32)
    b_off = np.asarray(b_off, np.float32)
    b_mask = np.asarray(b_mask, np.float32)
    w_dcn = np.asarray(w_dcn, np.float32)
    b_dcn = np.asarray(b_dcn, np.float32)

    wcat = np.zeros((73, C, K, K), np.float32)
    wcat[0:9] = w_off[0::2]
    wcat[32:41] = w_off[1::2]
    wcat[64:73] = w_mask
    w_om = np.zeros((128, KK, CG, 73), BF16)
    for tap in range(KK):
        ky, kx = tap // K, tap % K
        for cg in range(CG):
            w_om[:, tap, cg] = (
                wcat[:, cg * 128:(cg + 1) * 128, ky, kx].T.astype(BF16))

    ky_t = np.repeat(np.arange(K), K).astype(np.float32)
    kx_t = np.tile(np.arange(K), K).astype(np.float32)
    b_om = np.zeros((73, 1), np.float32)
    b_om[0:9, 0] = b_off[0::2] + ky_t - 1 + PAD
    b_om[32:41, 0] = b_off[1::2] + kx_t - 1 + PAD
    b_om[64:73, 0] = b_mask

    # doubled-contraction layout: XBAR'd h2 block b = (tap=b//4,
    # 64-chan segment b%4), partition q = 2*(c - 64*(b%4)) + x-corner;
    # both x-corners multiply the same weight (their sum = the sample)
    w_mm = np.zeros((128, 4 * KK, OG, 128), BF16)
    for tap in range(KK):
        ky, kx = tap // K, tap % K
        for seg in range(4):
            b = tap * 4 + seg
            cs = seg * 64
            wblk = w_dcn[:, cs:cs + 64, ky, kx].astype(BF16)  # [O, 64]
            for og in range(OG):
                wo = wblk[og * 128:(og + 1) * 128].T  # [64, 128]
                w_mm[0::2, b, og] = wo
                w_mm[1::2, b, og] = wo
    b_o = b_dcn.reshape(OG, 128, 1).transpose(1, 0, 2).copy()

    hio = np.broadcast_to(np.arange(HH, dtype=np.float32)[None, :],
                          (KK, HH)).astype(BF16)
    wio = np.broadcast_to(np.arange(W, dtype=np.float32)[None, :],
                          (KK, W)).astype(BF16)

    shared = dict(w_om=w_om, b_om=b_om, w_mm=w_mm, b_o=b_o, hio=hio, wio=wio)

    in_maps = []
    for core in range(NCORES):
        b, half = core // 2, core % 2
        h0 = half * HH
        xp = np.zeros((C, HP, WP), np.float32)
        glo, ghi = h0 - PAD, h0 + HH + PAD
        slo, shi = max(glo, 0), min(ghi, H)
        xp[:, slo - glo: slo - glo + (shi - slo), PAD:PAD + W] = x[b, :, slo:shi, :]
        xbf = xp.astype(BF16)
        x_cm = np.ascontiguousarray(xbf.reshape(CG, 128, HP, WP))
        xT = np.ascontiguousarray(xbf.reshape(C, L).T)  # [L, C]
        xT4c = np.zeros((4, L, C), BF16)
        xT4c[0] = xT
        xT4c[1, :L - 1] = xT[1:]
        xT4c[2, :L - WP] = xT[WP:]
        xT4c[3, :L - WP - 1] = xT[WP + 1:]
        # interleave corners per channel: row l = [c0 x 4 corners, c1 x ...]
        xT4 = np.ascontiguousarray(
            xT4c.transpose(1, 2, 0).reshape(L, 4 * C))
        im = dict(shared)
        im["x_cm"] = x_cm
        im["xT4"] = xT4
        in_maps.append(im)
    return in_maps


_NC_CACHE = {}


def kernel(**inputs):
    if "nc" not in _NC_CACHE:
        _NC_CACHE["nc"] = build_nc()
    nc = _NC_CACHE["nc"]
    in_maps = host_prep(**inputs)
    res = bass_utils.run_bass_kernel_spmd(nc, in_maps,
                                          core_ids=list(range(NCORES)))
    out = np.zeros((B, O, H, W), np.float32)
    for core in range(NCORES):
        b, half = core // 2, core % 2
        yv = np.asarray(res.results[core]["y"], np.float32)
        # un-sigma: column t*128 + a*16 + p holds position t*128 + p*8 + a
        yv = yv.reshape(O, NT, 8, 16).transpose(0, 1, 3, 2).reshape(O, HH, W)
        out[b, :, half * HH:(half + 1) * HH, :] = yv
    return out


# revision 32
# speedup vs baseline: 1.0532x; 1.0189x over previous
# DCNv2 (modulated deformable conv) Trainium2 Bass kernel.
#
# Sharding: pure data parallel over 8 cores; core = (batch, H-half), each
# core computes a (256, 32, 64) output slab from a zero-padded input slab.
#
# Per-core pipeline (preamble split in 2 halves interleaved with the
# tile loop):
#   1. offset/mask 3x3 conv on the PE (bf16 matmuls, fp32 PSUM, fused
#      bias (+tap/pad constants) and sigmoid on the ACT engine)
#   2. sampling fields (py/px/floor/frac/bilinear corner weights incl.
#      mask) on DVE in bf16; weights sigma-reordered
#   3. gather: SWDGE dma_gather from an HBM-resident 4-corner row table
#      (row l = [x[l], x[l+1], x[l+WP], x[l+WP+1]], 2KB) - ONE descriptor
#      per (tap, position)
#   4. 4-corner weighted combine split across engines: DVE multiplies
#      corners 0,1 (batched broadcast-AP op), ACT scales corners 2,3
#      in-place (per-partition scale), DVE does the 2-level add tree
#   5. colsT -> cols via a single XBAR DMA transpose per tile
#   6. main contraction out[o,hw] = sum_{c,p} w[o,c,p] cols[c,p,hw] as
#      bf16 matmuls accumulating in PSUM; bias on the ACT copy, un-sigma
#      in the output DMA APs.
import numpy as np
import ml_dtypes

import concourse.bass as bass
import concourse.mybir as mybir
from concourse import bacc
import concourse.tile as tile
from concourse import library_config
from concourse.masks import make_identity
from concourse import bass_utils

BF16 = ml_dtypes.bfloat16

B, C, H, W = 4, 256, 64, 64
O, K = 256, 3
KK = K * K
NCORES = 8
HH = H // 2            # 32 output rows per core
PAD = 5                # zero halo; requires |offset| < PAD - 1
HP, WP = 48, 80        # padded local input dims
L = HP * WP            # 3840 source locations
NHW = HH * W           # 2048 output positions per core
NT = NHW // 128        # 16 gather tiles
NHWH = NHW // 2        # positions per preamble half
CG = C // 128
OG = O // 128
A = mybir.AluOpType
ACTF = mybir.ActivationFunctionType
FP32 = mybir.dt.float32
BF = mybir.dt.bfloat16
I16 = mybir.dt.int16

MAGIC = float(np.float32(2 ** 23))


def build_nc():
    nc = bacc.Bacc("TRN2", target_bir_lowering=False, num_devices=NCORES,
                   num_swdge_queues=2)

    x_cm_d = nc.dram_tensor("x_cm", [CG, 128, HP, WP], BF, kind="ExternalInput").ap()
    xT4_d = nc.dram_tensor("xT4", [L, 4 * C], BF, kind="ExternalInput").ap()
    w_om_d = nc.dram_tensor("w_om", [128, KK, CG, 73], BF, kind="ExternalInput").ap()
    b_om_d = nc.dram_tensor("b_om", [73, 1], FP32, kind="ExternalInput").ap()
    w_mm_d = nc.dram_tensor("w_mm", [128, 4 * KK, OG, 128], BF,
                            kind="ExternalInput").ap()
    b_o_d = nc.dram_tensor("b_o", [128, OG, 1], FP32, kind="ExternalInput").ap()
    hio_d = nc.dram_tensor("hio", [KK, HH], BF, kind="ExternalInput").ap()
    wio_d = nc.dram_tensor("wio", [KK, W], BF, kind="ExternalInput").ap()
    y_d = nc.dram_tensor("y", [OG, 128, NHW], BF, kind="ExternalOutput").ap()

    with tile.TileContext(nc) as tc:
        with (
            tc.tile_pool(name="const", bufs=1) as const,
            tc.tile_pool(name="persist", bufs=1) as persist,
            tc.tile_pool(name="dramp", bufs=1, space="DRAM") as dramp,
            tc.tile_pool(name="fld", bufs=1) as fld,
            tc.tile_pool(name="cwp", bufs=16) as cwp,
            tc.tile_pool(name="qp", bufs=3) as qp,
            tc.tile_pool(name="hp", bufs=2) as hp,
            tc.tile_pool(name="colsp", bufs=2) as colsp,
            tc.tile_pool(name="outp", bufs=2) as outp,
            tc.tile_pool(name="ps_conv", bufs=1, space="PSUM") as ps_conv,
            tc.tile_pool(name="ps_cw", bufs=2, space="PSUM") as ps_cw,
            tc.tile_pool(name="ps_m", bufs=4, space="PSUM") as ps_m,
        ):
            # ---- constants into SBUF ----
            w_om = const.tile([128, KK, CG, 73], BF)
            nc.sync.dma_start(w_om[:], w_om_d)
            w_mm = const.tile([128, 4 * KK, OG, 128], BF)
            nc.scalar.dma_start(w_mm[:], w_mm_d)
            b_om = const.tile([73, 1], FP32)
            nc.sync.dma_start(b_om[:], b_om_d)
            b_o = const.tile([128, OG, 1], FP32)
            nc.sync.dma_start(b_o[:], b_o_d)
            hio2 = const.tile([KK, HH], BF)
            nc.sync.dma_start(hio2[:], hio_d)
            wio2 = const.tile([KK, W], BF)
            nc.sync.dma_start(wio2[:], wio_d)
            idbf = const.tile([128, 128], BF)
            make_identity(nc, idbf[:])
            nc.gpsimd.load_library(library_config.mlp)

            idx16 = persist.tile([KK, NHW], I16)
            # bilinear corner weights, sigma-reordered, bf16
            w00s = persist.tile([KK, NHW], BF)
            w01s = persist.tile([KK, NHW], BF)
            w10s = persist.tile([KK, NHW], BF)
            w11s = persist.tile([KK, NHW], BF)
            wrapped = persist.tile([128, NT, 72], I16)
            db = dramp.tile([16, NT, 72], I16, name="db")

            # x slab rows 4..34 are all the conv reads (out rows 0..31,
            # taps 0..2, offset +4-1); two half tiles of 15+16 rows
            x_sb = []
            for cg in range(CG):
                xa = fld.tile([128, 19, WP], BF, name=f"xsbA{cg}")
                nc.sync.dma_start(xa[:], x_cm_d[cg][:, 4:23])
                xb = fld.tile([128, 19, WP], BF, name=f"xsbB{cg}")
                nc.scalar.dma_start(xb[:], x_cm_d[cg][:, 20:39])
                x_sb.append((xa, xb))

            corners = [w00s, w01s, w10s, w11s]

            def emit_half(hf):
                """offset/mask conv + sampling fields + idx bounce for
                output rows [hf*16, hf*16+16) = tiles [8hf, 8hf+8).
                Returns a list of emission chunks to interleave."""
                cols = slice(hf * NHWH, (hf + 1) * NHWH)
                st = {}

                def c_alloc():
                    st["offy"] = fld.tile([KK, NHWH], FP32, tag="t_f32",
                                          bufs=3, name=f"offy{hf}")
                    st["offx"] = fld.tile([KK, NHWH], FP32, tag="t_f32",
                                          bufs=3, name=f"offx{hf}")
                    st["msk"] = fld.tile([KK, NHWH], BF, tag="t_msk", bufs=2,
                                         name=f"msk{hf}")

                def c_conv(bl):
                    offy_h, offx_h, msk = st["offy"], st["offx"], st["msk"]
                    bk = hf * 4 + bl
                    ps = ps_conv.tile([73, 4 * WP], FP32, tag="psc")
                    n = 0
                    for cg in range(CG):
                        for tap in range(KK):
                            ky, kx = tap // K, tap % K
                            r = bk * 4 + ky - 16 * hf
                            xt = x_sb[cg][hf]
                            rhs = xt[:, r, 4 + kx:]
                            rhs = bass.AP(tensor=rhs.tensor, offset=rhs.offset,
                                          ap=[rhs.ap[0], [1, 4 * WP]])
                            nc.tensor.matmul(ps[:], w_om[:, tap, cg], rhs,
                                             start=(n == 0),
                                             stop=(n == 2 * KK - 1))
                            n += 1
                    sl = slice(bl * 4 * W, (bl + 1) * 4 * W)
                    psv = [None, None, None]
                    for i, base in enumerate((0, 32, 64)):
                        p4 = ps[base:base + 9].rearrange("c (r x) -> c r x",
                                                         r=4)
                        psv[i] = p4[:, :, 0:W]
                    nc.scalar.activation(offy_h[:, sl], psv[0], ACTF.Identity,
                                         bias=b_om[0:9])
                    nc.scalar.activation(offx_h[:, sl], psv[1], ACTF.Identity,
                                         bias=b_om[32:41])
                    nc.scalar.activation(msk[:, sl], psv[2], ACTF.Sigmoid,
                                         bias=b_om[64:73])

                def c_fields_a():
                    offy_h, offx_h = st["offy"], st["offx"]
                    # sampling positions: py = offy + h, px = offx + w
                    # (broadcast row/col index via stride-0 APs)
                    py = fld.tile([KK, NHWH], FP32, tag="t_f32", bufs=3,
                                  name="py")
                    px = fld.tile([KK, NHWH], FP32, tag="t_f32", bufs=3,
                                  name="px")
                    st["py"], st["px"] = py, px
                    hbc = bass.AP(tensor=hio2.tensor,
                                  offset=hio2.offset + hf * 16,
                                  ap=[hio2[:].ap[0], [1, 16], [0, W]])
                    wbc2 = bass.AP(tensor=wio2.tensor, offset=wio2.offset,
                                   ap=[wio2[:].ap[0], [0, 16], [1, W]])
                    ov = offy_h[:].rearrange("c (h w) -> c h w", h=16)
                    xv = offx_h[:].rearrange("c (h w) -> c h w", h=16)
                    nc.vector.tensor_tensor(
                        py[:].rearrange("c (h w) -> c h w", h=16), ov, hbc,
                        A.add)
                    nc.vector.tensor_tensor(
                        px[:].rearrange("c (h w) -> c h w", h=16), xv, wbc2,
                        A.add)

                    def floor_clamp(dst, src, hi):
                        # dst = floor(src) via round(src-0.5); offsets are
                        # bounded (|off|<4 asserted) so no clamp is needed
                        t1 = fld.tile([KK, NHWH], FP32, tag="t_fc", bufs=1,
                                      name="fc1")
                        nc.vector.tensor_scalar(t1[:], src[:], MAGIC - 0.5,
                                                None, A.add)
                        nc.vector.tensor_scalar(dst[:], t1[:], MAGIC, None,
                                                A.subtract)

                    y0 = fld.tile([KK, NHWH], BF, tag="t_bf", bufs=3,
                                  name="y0")
                    x0 = fld.tile([KK, NHWH], BF, tag="t_bf", bufs=3,
                                  name="x0")
                    st["y0"], st["x0"] = y0, x0
                    floor_clamp(y0, py, HP - 2)
                    floor_clamp(x0, px, WP - 2)
                    nc.vector.scalar_tensor_tensor(idx16[:, cols], y0[:],
                                                   float(WP), x0[:],
                                                   A.mult, A.add)

                def c_fields_b():
                    py, px = st["py"], st["px"]
                    y0, x0, msk = st["y0"], st["x0"], st["msk"]
                    fy = fld.tile([KK, NHWH], BF, tag="t_bf", bufs=3,
                                  name="fy")
                    nc.vector.tensor_tensor(fy[:], py[:], y0[:], A.subtract)
                    fx = fld.tile([KK, NHWH], BF, tag="t_bf", bufs=3,
                                  name="fx")
                    nc.vector.tensor_tensor(fx[:], px[:], x0[:], A.subtract)

                    u = fld.tile([KK, NHWH], BF, tag="t_ug", bufs=2, name="u")
                    gy = fld.tile([KK, NHWH], BF, tag="t_ug", bufs=2,
                                  name="gy")
                    nc.vector.tensor_tensor(u[:], fy[:], msk[:], A.mult)
                    nc.vector.tensor_tensor(gy[:], msk[:], u[:], A.subtract)

                    # w01 = gy*fx; w00 = gy-w01; w11 = u*fx; w10 = u-w11
                    wna = fld.tile([KK, NHWH], BF, tag="t_wn", bufs=2,
                                   name="wna")
                    wnb = fld.tile([KK, NHWH], BF, tag="t_wn", bufs=2,
                                   name="wnb")

                    def sigma_store(wdst, wsrc):
                        src = wsrc[:].rearrange("c (t p a) -> c t p a", t=8,
                                                p=16).transpose([0, 1, 3, 2])
                        nc.vector.tensor_copy(wdst[:, cols], src)

                    nc.vector.tensor_tensor(wna[:], gy[:], fx[:], A.mult)
                    nc.vector.tensor_tensor(wnb[:], gy[:], wna[:], A.subtract)
                    sigma_store(w01s, wna)
                    sigma_store(w00s, wnb)
                    wnc = fld.tile([KK, NHWH], BF, tag="t_wn", bufs=2,
                                   name="wnc")
                    wnd = fld.tile([KK, NHWH], BF, tag="t_wn", bufs=2,
                                   name="wnd")
                    nc.vector.tensor_tensor(wnc[:], u[:], fx[:], A.mult)
                    nc.vector.tensor_tensor(wnd[:], u[:], wnc[:], A.subtract)
                    sigma_store(w11s, wnc)
                    sigma_store(w10s, wnd)

                def c_cw():
                    # per-position corner weights for this half's 8 tiles
                    # (hoisted off the per-tile path so the ACT queue never
                    # blocks DVE's combine on a stalled out-copy)
                    for tt in range(8):
                        t = hf * 8 + tt
                        cw4T = cwp.tile([128, KK, 4], BF, tag="cw4T",
                                        name="cw4T")
                        cw_all[t] = cw4T
                        for j, wf in enumerate(corners):
                            psf = ps_cw.tile([128, KK], BF, tag="psf",
                                             name="psf")
                            nc.tensor.transpose(
                                psf[:], wf[0:9, t * 128:(t + 1) * 128],
                                idbf[0:9, 0:9])
                            dst = bass.AP(tensor=cw4T.tensor,
                                          offset=cw4T.offset + j,
                                          ap=[cw4T[:].ap[0], [4, KK]])
                            nc.scalar.activation(dst, psf[:], ACTF.Identity)

                def c_bounce():
                    # idx -> DRAM bounce (partition-major) -> replicated
                    for tt in range(8):
                        t = hf * 8 + tt
                        dst_ap = bass.AP(tensor=db.tensor,
                                         offset=db.offset + t * 72,
                                         ap=[[8, KK], [NT * 72, 16], [1, 8]])
                        src_w = idx16[:, t * 128:(t + 1) * 128].rearrange(
                            "b (p q) -> b p q", p=16)
                        nc.sync.dma_start(dst_ap, src_w)
                    rep_ap = bass.AP(tensor=db.tensor,
                                     offset=db.offset + hf * 8 * 72,
                                     ap=[[0, 8], [NT * 72, 16], [1, 8 * 72]])
                    wdst = wrapped[:].rearrange("p t q -> p (t q)")
                    nc.sync.dma_start(wdst[:, hf * 576:(hf + 1) * 576],
                                      rep_ap)

                def c_cw_bounce():
                    c_cw()
                    c_bounce()

                return [c_alloc, lambda: c_conv(0), lambda: c_conv(1),
                        lambda: c_conv(2), lambda: c_conv(3),
                        c_fields_a, c_fields_b, c_cw_bounce]

            def emit_tile(t, cols_sb):
                cw4T = cw_all[t]
                # gather Q4[128(sig hw), 9(tap), 1024(4 corners x 256c)]
                Q4 = qp.tile([128, KK, C, 4], BF, tag="Q4", name="Q4")
                nc.gpsimd.dma_gather(
                    out_ap=Q4[:].rearrange("p k c j -> p k (c j)"),
                    in_ap=xT4_d, idxs_ap=wrapped[:, t],
                    num_idxs=KK * 128, num_idxs_reg=KK * 128,
                    elem_size=4 * C, single_packet=False, queue_num=t % 2,
                )

                # weighted 4-corner combine, all on DVE with stride-1 last
                # dims (keeps the 16-bit 2x fast path): corners innermost
                p0 = Q4[:].ap[0]
                wbc = bass.AP(tensor=cw4T.tensor, offset=cw4T.offset,
                              ap=[cw4T[:].ap[0], [4, KK], [0, C], [1, 4]])
                nc.vector.tensor_tensor(Q4[:], Q4[:], wbc, A.mult)

                # add tree: h2[.,c,y'] = y-pairs; colsT = x-pairs
                h2 = hp.tile([128, KK, C, 2], BF, tag="h", name="h")
                qe = bass.AP(tensor=Q4.tensor, offset=Q4.offset,
                             ap=[p0, [4 * C, KK], [4, C], [1, 2]])
                qo = bass.AP(tensor=Q4.tensor, offset=Q4.offset + 2,
                             ap=[p0, [4 * C, KK], [4, C], [1, 2]])
                nc.vector.tensor_tensor(h2[:], qe, qo, A.add)

                # transpose h2 to cols2 [128(2c+j), 36 blk, 256(sig)]
                # via XBAR; the x-corner sum rides the main contraction
                csrc = bass.AP(tensor=h2.tensor, offset=h2.offset,
                               ap=[h2[:].ap[0], [1, KK * C * 2]])
                cdst = bass.AP(tensor=cols_sb.tensor,
                               offset=cols_sb.offset + (t % 2) * 128,
                               ap=[cols_sb[:].ap[0], [256, 4 * KK], [1, 128]])
                nc.sync.dma_start(cdst, csrc, transpose=True)

            def emit_group(g, cols_sb, ps_out):
                # main contraction for tiles 2g..2g+1; result parked in
                # PSUM -- the bias+copy+store runs after the tile loop so
                # the ACT queue never head-of-line-blocks the combine
                for og in range(OG):
                    psO = ps_m.tile([128, 256], FP32, tag=f"psO{g % 2}",
                                    bufs=2, name="psO")
                    ps_out[g * OG + og] = psO
                    for b in range(4 * KK):
                        nc.tensor.matmul(
                            psO[:], w_mm[:, b, og], cols_sb[:, b],
                            start=(b == 0), stop=(b == 4 * KK - 1))

            def emit_store(g, ps_out):
                for og in range(OG):
                    psO = ps_out[g * OG + og]
                    out_t = outp.tile([128, 256], BF, tag="out", name="out_t")
                    nc.scalar.activation(out_t[:], psO[:], ACTF.Identity,
                                         bias=b_o[:, og])
                    # stored in sigma order; host un-permutes
                    nc.scalar.dma_start(y_d[og, :, g * 256:(g + 1) * 256],
                                        out_t[:])

            cw_all = [None] * NT
            ps_out = [None] * (8 * OG)
            # ---- emission: both preamble halves upfront (interleaving
            # them into the loop caused 30us mid-loop pipeline bubbles) ----
            for c in emit_half(0):
                c()
            for c in emit_half(1):
                c()
            cols_sb = None
            for t in range(NT):
                if t % 2 == 0:
                    cols_sb = colsp.tile([128, 4 * KK, 256], BF, tag="cols",
                                         name="cols")
                emit_tile(t, cols_sb)
                if t % 2 == 1:
                    g = t // 2
                    emit_group(g, cols_sb, ps_out)
                    if g > 0:
                        emit_store(g - 1, ps_out)
                    if g == 7:
                        emit_store(7, ps_out)
    nc.compile()
    return nc


# ---------------- host side ----------------

def host_prep(x, w_off, b_off, w_mask, b_mask, w_dcn, b_dcn):
    """Build the 8 per-core input maps (pure layout prep / sharding)."""
    x = np.asarray(x, np.float32)
    w_off = np.asarray(w_off, np.float32)
    w_mask = np.asarray(w_mask, np.float32)
    b_off = np.asarray(b_off, np.float32)
    b_mask = np.asarray(b_mask, np.float32)
    w_dcn = np.asarray(w_dcn, np.float32)
    b_dcn = np.asarray(b_dcn, np.float32)

    wcat = np.zeros((73, C, K, K), np.float32)
    wcat[0:9] = w_off[0::2]
    wcat[32:41] = w_off[1::2]
    wcat[64:73] = w_mask
    w_om = np.zeros((128, KK, CG, 73), BF16)
    for tap in range(KK):
        ky, kx = tap // K, tap % K
        for cg in range(CG):
            w_om[:, tap, cg] = (
                wcat[:, cg * 128:(cg + 1) * 128, ky, kx].T.astype(BF16))

    ky_t = np.repeat(np.arange(K), K).astype(np.float32)
    kx_t = np.tile(np.arange(K), K).astype(np.float32)
    b_om = np.zeros((73, 1), np.float32)
    b_om[0:9, 0] = b_off[0::2] + ky_t - 1 + PAD
    b_om[32:41, 0] = b_off[1::2] + kx_t - 1 + PAD
    b_om[64:73, 0] = b_mask

    # doubled-contraction layout: XBAR'd h2 block b = (tap=b//4,
    # 64-chan segment b%4), partition q = 2*(c - 64*(b%4)) + x-corner;
    # both x-corners multiply the same weight (their sum = the sample)
    w_mm = np.zeros((128, 4 * KK, OG, 128), BF16)
    for tap in range(KK):
        ky, kx = tap // K, tap % K
        for seg in range(4):
            b = tap * 4 + seg
            cs = seg * 64
            wblk = w_dcn[:, cs:cs + 64, ky, kx].astype(BF16)  # [O, 64]
            for og in range(OG):
                wo = wblk[og * 128:(og + 1) * 128].T  # [64, 128]
                w_mm[0::2, b, og] = wo
                w_mm[1::2, b, og] = wo
    b_o = b_dcn.reshape(OG, 128, 1).transpose(1, 0, 2).copy()

    hio = np.broadcast_to(np.arange(HH, dtype=np.float32)[None, :],
                          (KK, HH)).astype(BF16)
    wio = np.broadcast_to(np.arange(W, dtype=np.float32)[None, :],
                          (KK, W)).astype(BF16)

    shared = dict(w_om=w_om, b_om=b_om, w_mm=w_mm, b_o=b_o, hio=hio, wio=wio)

    in_maps = []
    for core in range(NCORES):
        b, half = core // 2, core % 2
        h0 = half * HH
        xp = np.zeros((C, HP, WP), np.float32)
        glo, ghi = h0 - PAD, h0 + HH + PAD
        slo, shi = max(glo, 0), min(ghi, H)
        xp[:, slo - glo: slo - glo + (shi - slo), PAD:PAD + W] = x[b, :, slo:shi, :]
        xbf = xp.astype(BF16)
        x_cm = np.ascontiguousarray(xbf.reshape(CG, 128, HP, WP))
        xT = np.ascontiguousarray(xbf.reshape(C, L).T)  # [L, C]
        xT4c = np.zeros((4, L, C), BF16)
        xT4c[0] = xT
        xT4c[1, :L - 1] = xT[1:]
        xT4c[2, :L - WP] = xT[WP:]
        xT4c[3, :L - WP - 1] = xT[WP + 1:]
        # interleave corners per channel: row l = [c0 x 4 corners, c1 x ...]
        xT4 = np.ascontiguousarray(
            xT4c.transpose(1, 2, 0).reshape(L, 4 * C))
        im = dict(shared)
        im["x_cm"] = x_cm
        im["xT4"] = xT4
        in_maps.append(im)
    return in_maps


_NC_CACHE = {}


def kernel(**inputs):
    if "nc" not in _NC_CACHE:
        _NC_CACHE["nc"] = build_nc()
    nc = _NC_CACHE["nc"]
    in_maps = host_prep(**inputs)
    res = bass_utils.run_bass_kernel_spmd(nc, in_maps,
                                          core_ids=list(range(NCORES)))
    out = np.zeros((B, O, H, W), np.float32)
    for core in range(NCORES):
        b, half = core // 2, core % 2
        yv = np.asarray(res.results[core]["y"], np.float32)
        # un-sigma: column t*128 + a*16 + p holds position t*128 + p*8 + a
        yv = yv.reshape(O, NT, 8, 16).transpose(0, 1, 3, 2).reshape(O, HH, W)
        out[b, :, half * HH:(half + 1) * HH, :] = yv
    return out
